# revision 10
# baseline (speedup 1.0000x reference)
"""Trainium2 Bass kernel for nn_DctAtt (B=32, D=1024, N=4096, K=5).

The reference computes, per (b, d) row of x:
    coeffs = x[b,d,:] @ C          (C: [N, K] DCT-II ortho, first K rows)
    att    = coeffs @ dw_w + dw_b
Both steps are linear in x, so they collapse into a single dot product
with the precomputed vector w = C @ dw_w:
    att[b,d] = x[b,d,:] . w + dw_b
The device kernel streams x through that dot product -- this is the
memory-bound part. The remaining work (BatchNorm over all B*D values,
GELU, scalar affine, softmax over D) touches only a [32, 1024] array
and runs on the host, using the exact global batch statistics.

v2 (this file): x is quantized to fp16 on the host (end-to-end absmax
rel-err 8.5e-4 vs the 2e-2 gate -- fp16's 10 mantissa bits are plenty
for a 4096-term unit-variance dot), halving HBM traffic vs f32:
32 MiB/core instead of 64 MiB. The DVE custom op used by the f32
version is locked to 1x perf mode, so at fp16 the contraction moves to
the TensorEngine instead: the host pre-transposes each core's shard to
x^T [N, rows] so the contraction dim lands on SBUF partitions, and PE
accumulates y[1, rows] = sum_t w_blk[t]^T @ xT_blk[t] into 8 PSUM
banks (512 rows each) over 32 K-blocks. PE busy ~60us < ~91us of DMA,
so the kernel stays at the fp16 memory roofline. w is pre-scaled by
256 (undone on host) to keep its small values in fp16 normal range.

Sharding: data-parallel over batch B across the 8 NeuronCores
(4 batches = 4096 rows of 4096 fp16 = 32 MiB per core).
"""

import math

import numpy as np

import concourse.bacc as bacc
import concourse.mybir as mybir
import concourse.tile as tile
from concourse import bass_utils

# Problem constants (hardcoded: the grading harness ships only this file).
B, D, N = 32, 1024, 4096
K = 5
BN_EPS = 1e-5
N_CORES = 8
P = 128
ROWS_PER_CORE = (B // N_CORES) * D  # 4096
NBLK = N // P  # 32 K-blocks of 128
FD = 512  # PSUM bank width in f32
NBANKS = ROWS_PER_CORE // FD  # 8
W_SCALE = 256.0  # keeps w (|w| in [7e-6, 0.015]) in fp16 normal range

# Tuning knobs (env overrides are for the dev harness only; defaults are
# what the graded kernel uses).
import os as _os

# J consecutive dram rows per SBUF partition -> J*8 KiB contiguous DMA
# descriptor lines (8 KiB lines measured 307 GB/s vs 370 at 16 KiB).
INTERLEAVE = int(_os.environ.get("DCT_J", "2"))
XP_BUFS = int(_os.environ.get("DCT_BUFS", "10"))  # in-flight x tiles
# Split the last tile's DMA into row-halves and run its matmuls
# bank-major so the per-bank accumulation stops (and the PSUM->SBUF
# copies behind them) pipeline instead of bunching after the last byte.
TAIL_SPLIT = int(_os.environ.get("DCT_TAILSPLIT", "1"))

_compiled_nc = None


def _build():
    """Build + compile the per-core Bass program (cached per process)."""
    global _compiled_nc
    if _compiled_nc is not None:
        return _compiled_nc

    nc = bacc.Bacc(
        "TRN2",
        target_bir_lowering=False,
        debug=False,
        enable_asserts=False,
        num_devices=N_CORES,
    )
    f32 = mybir.dt.float32
    f16 = mybir.dt.float16
    J = INTERLEAVE
    xT = nc.dram_tensor("xT", [N, ROWS_PER_CORE], f16, kind="ExternalInput").ap()
    w_in = nc.dram_tensor("w_pk", [P, NBLK], f16, kind="ExternalInput").ap()
    y_out = nc.dram_tensor("y_out", [1, ROWS_PER_CORE], f32, kind="ExternalOutput").ap()

    with tile.TileContext(nc) as tc:
        with (
            tc.tile_pool(name="wp", bufs=1) as wp,
            tc.tile_pool(name="xp", bufs=XP_BUFS) as xp,
            tc.tile_pool(name="yp", bufs=1) as yp,
            tc.tile_pool(name="ps", bufs=1, space="PSUM") as ps,
        ):
            # w on the scalar HWDGE ring so it can't head-of-line block
            # the x stream on the SP ring.
            w_sb = wp.tile([P, NBLK], f16)
            nc.scalar.dma_start(out=w_sb, in_=w_in)
            y_sb = yp.tile([1, ROWS_PER_CORE], f32)
            # One persistent PSUM bank per 512-row output chunk.
            accs = [ps.tile([1, FD], f32, name=f"acc{b}") for b in range(NBANKS)]
            # xv[T, p, j, r] = xT row (T*128 + p)*J + j -- partition p holds
            # J consecutive dram rows, i.e. one contiguous J*8 KiB line.
            # w_pk is packed on the host to the same (p, j) -> n mapping.
            xv = xT.rearrange("(T p j) r -> T p j r", p=P, j=J)
            NT = NBLK // J
            RH = ROWS_PER_CORE // 2
            for T in range(NT):
                # Alternate x tiles between the two HWDGE rings (SP + Act)
                # so descriptor generation runs 2-wide: one ring's ~0.7us
                # per-DMA issue latency starves the 16 SDMA engines during
                # the cold-start ramp.
                ring = nc.sync if T % 2 == 0 else nc.scalar
                xt = xp.tile([P, J, ROWS_PER_CORE], f16)
                if TAIL_SPLIT and T == NT - 1:
                    # Row-halves: banks 0..3 only need the first half, so
                    # their stop-matmuls (and copies) overlap half B's DMA.
                    nc.sync.dma_start(out=xt[:, :, :RH], in_=xv[T][:, :, :RH])
                    nc.scalar.dma_start(out=xt[:, :, RH:], in_=xv[T][:, :, RH:])
                    for b in range(NBANKS):
                        for j in range(J):
                            t = T * J + j
                            nc.tensor.matmul(
                                accs[b],
                                lhsT=w_sb[:, t : t + 1],
                                rhs=xt[:, j, b * FD : (b + 1) * FD],
                                start=(t == 0),
                                stop=(t == NBLK - 1),
                            )
                        # Copy as soon as this bank's accumulation stops,
                        # alternating engines so copies pipeline 2-wide.
                        eng = nc.vector if b % 2 else nc.scalar
                        if b % 2:
                            eng.tensor_copy(
                                y_sb[:, b * FD : (b + 1) * FD], accs[b]
                            )
                        else:
                            eng.copy(
                                out=y_sb[:, b * FD : (b + 1) * FD], in_=accs[b]
                            )
                    continue
                ring.dma_start(out=xt, in_=xv[T])
                for j in range(J):
                    t = T * J + j
                    for b in range(NBANKS):
                        # acc[b][0, r] += sum_p w_pk[p, t] * xt[p, j, b*FD+r]
                        nc.tensor.matmul(
                            accs[b],
                            lhsT=w_sb[:, t : t + 1],
                            rhs=xt[:, j, b * FD : (b + 1) * FD],
                            start=(t == 0),
                            stop=(t == NBLK - 1),
                        )
            if not TAIL_SPLIT:
                for b in range(NBANKS):
                    nc.scalar.copy(out=y_sb[:, b * FD : (b + 1) * FD], in_=accs[b])
            nc.sync.dma_start(out=y_out, in_=y_sb)

    nc.compile()
    _compiled_nc = nc
    return nc


def _dct_weight(dw_w):
    """w = C @ dw_w in float64, where C is the [N, K] ortho DCT-II basis."""
    n = np.arange(N, dtype=np.float64)
    k = np.arange(K, dtype=np.float64)
    C = np.cos(np.pi * (2.0 * n[:, None] + 1.0) * k[None, :] / (2.0 * N))
    C *= math.sqrt(2.0 / N)
    C[:, 0] *= 1.0 / math.sqrt(2.0)
    return (C @ np.asarray(dw_w, dtype=np.float64)).astype(np.float32)


def _erf(x):
    try:
        from scipy.special import erf

        return erf(x)
    except Exception:
        return np.vectorize(math.erf)(x).astype(x.dtype)


def _run_device(inputs, trace=False, **spmd_kwargs):
    """Run the dot-product phase on the 8 cores; return att [B, D] (pre-BN,
    pre-bias) and the BassKernelResults (for profiling from harnesses)."""
    x = np.asarray(inputs["x"])
    w = _dct_weight(inputs["dw_w"])
    w16 = (w * np.float32(W_SCALE)).astype(np.float16)
    # w_pk[p, T*J + j] = w[(T*128 + p)*J + j], matching the xv interleave.
    J = INTERLEAVE
    w_pk = np.ascontiguousarray(
        w16.reshape(NBLK // J, P, J).transpose(1, 0, 2).reshape(P, NBLK)
    )

    nc = _build()
    b_per_core = B // N_CORES
    in_maps = []
    for c in range(N_CORES):
        xs = x[c * b_per_core : (c + 1) * b_per_core].reshape(ROWS_PER_CORE, N)
        xTc = np.ascontiguousarray(xs.astype(np.float16).T)  # [N, rows]
        in_maps.append({"xT": xTc, "w_pk": w_pk})

    res = bass_utils.run_bass_kernel_spmd(
        nc, in_maps, core_ids=list(range(N_CORES)), trace=trace, **spmd_kwargs
    )
    att = np.concatenate(
        [res.results[c]["y_out"].reshape(-1) for c in range(N_CORES)]
    )
    att = (att.astype(np.float32) / np.float32(W_SCALE)).reshape(B, D)
    return att, res


def _postprocess(att, inputs):
    """Host tail on the tiny [B, D] array: +dw_b, BatchNorm (global batch
    stats, training mode), exact GELU, 1x1 conv affine, softmax over D."""
    dw_b = np.float32(np.asarray(inputs["dw_b"]).reshape(-1)[0])
    gamma = np.float32(np.asarray(inputs["gamma"]).reshape(-1)[0])
    beta = np.float32(np.asarray(inputs["beta"]).reshape(-1)[0])
    conv_w = np.float32(np.asarray(inputs["conv_w"]).reshape(-1)[0])
    conv_b = np.float32(np.asarray(inputs["conv_b"]).reshape(-1)[0])

    att = att.astype(np.float32) + dw_b
    mean = att.mean(dtype=np.float64)
    var = np.mean((att.astype(np.float64) - mean) ** 2)
    inv_std = np.float32(1.0 / math.sqrt(var + BN_EPS))
    att = (att - np.float32(mean)) * inv_std * gamma + beta
    # Exact GELU: x * 0.5 * (1 + erf(x / sqrt(2)))
    att = (att * 0.5 * (1.0 + _erf(att / np.float32(math.sqrt(2.0))))).astype(
        np.float32
    )
    att1 = att * conv_w + conv_b
    att1 = att1 - att1.max(axis=-1, keepdims=True)
    e = np.exp(att1.astype(np.float32))
    att1 = (e / e.sum(axis=-1, keepdims=True)).astype(np.float32)
    att1 = att1[:, :, None]
    return att1, (np.float32(1.0) - att1).astype(np.float32)


def kernel(**inputs):
    att, _ = _run_device(inputs)
    return _postprocess(att, inputs)


# revision 12
# speedup vs baseline: 1.0329x; 1.0329x over previous
"""Trainium2 Bass kernel for nn_DctAtt (B=32, D=1024, N=4096, K=5).

The reference computes, per (b, d) row of x:
    coeffs = x[b,d,:] @ C          (C: [N, K] DCT-II ortho, first K rows)
    att    = coeffs @ dw_w + dw_b
Both steps are linear in x, so they collapse into a single dot product
with the precomputed vector w = C @ dw_w:
    att[b,d] = x[b,d,:] . w + dw_b
The device kernel streams x through that dot product -- this is the
memory-bound part. The remaining work (BatchNorm over all B*D values,
GELU, scalar affine, softmax over D) touches only a [32, 1024] array
and runs on the host, using the exact global batch statistics.

v2 (this file): x is quantized to fp16 on the host (end-to-end absmax
rel-err 8.5e-4 vs the 2e-2 gate -- fp16's 10 mantissa bits are plenty
for a 4096-term unit-variance dot), halving HBM traffic vs f32:
32 MiB/core instead of 64 MiB. The DVE custom op used by the f32
version is locked to 1x perf mode, so at fp16 the contraction moves to
the TensorEngine instead: the host pre-transposes each core's shard to
x^T [N, rows] so the contraction dim lands on SBUF partitions, and PE
accumulates y[1, rows] = sum_t w_blk[t]^T @ xT_blk[t] into 8 PSUM
banks (512 rows each) over 32 K-blocks. PE busy ~60us < ~91us of DMA,
so the kernel stays at the fp16 memory roofline. w is pre-scaled by
256 (undone on host) to keep its small values in fp16 normal range.

Sharding: data-parallel over batch B across the 8 NeuronCores
(4 batches = 4096 rows of 4096 fp16 = 32 MiB per core).
"""

import math

import numpy as np

import concourse.bacc as bacc
import concourse.mybir as mybir
import concourse.tile as tile
from concourse import bass_utils

# Problem constants (hardcoded: the grading harness ships only this file).
B, D, N = 32, 1024, 4096
K = 5
BN_EPS = 1e-5
N_CORES = 8
P = 128
ROWS_PER_CORE = (B // N_CORES) * D  # 4096
NBLK = N // P  # 32 K-blocks of 128
FD = 512  # PSUM bank width in f32
NBANKS = ROWS_PER_CORE // FD  # 8
W_SCALE = 256.0  # keeps w (|w| in [7e-6, 0.015]) in fp16 normal range

# Tuning knobs (env overrides are for the dev harness only; defaults are
# what the graded kernel uses).
import os as _os

# J consecutive dram rows per SBUF partition -> J*8 KiB contiguous DMA
# descriptor lines (8 KiB lines measured 307 GB/s vs 370 at 16 KiB).
INTERLEAVE = int(_os.environ.get("DCT_J", "2"))
XP_BUFS = int(_os.environ.get("DCT_BUFS", "10"))  # in-flight x tiles
# Split the last tile's DMA into row-halves and run its matmuls
# bank-major so the per-bank accumulation stops (and the PSUM->SBUF
# copies behind them) pipeline instead of bunching after the last byte.
TAIL_SPLIT = int(_os.environ.get("DCT_TAILSPLIT", "1"))

_compiled_nc = None


def _build():
    """Build + compile the per-core Bass program (cached per process)."""
    global _compiled_nc
    if _compiled_nc is not None:
        return _compiled_nc

    nc = bacc.Bacc(
        "TRN2",
        target_bir_lowering=False,
        debug=False,
        enable_asserts=False,
        num_devices=N_CORES,
    )
    f32 = mybir.dt.float32
    f16 = mybir.dt.float16
    J = INTERLEAVE
    xT = nc.dram_tensor("xT", [N, ROWS_PER_CORE], f16, kind="ExternalInput").ap()
    w_in = nc.dram_tensor("w_pk", [P, NBLK], f16, kind="ExternalInput").ap()
    y_out = nc.dram_tensor("y_out", [1, ROWS_PER_CORE], f32, kind="ExternalOutput").ap()

    with tile.TileContext(nc) as tc:
        with (
            tc.tile_pool(name="wp", bufs=1) as wp,
            tc.tile_pool(name="xp", bufs=XP_BUFS) as xp,
            tc.tile_pool(name="yp", bufs=1) as yp,
            tc.tile_pool(name="ps", bufs=1, space="PSUM") as ps,
        ):
            # w on the scalar HWDGE ring so it can't head-of-line block
            # the x stream on the SP ring.
            w_sb = wp.tile([P, NBLK], f16)
            nc.scalar.dma_start(out=w_sb, in_=w_in)
            y_sb = yp.tile([1, ROWS_PER_CORE], f32)
            # One persistent PSUM bank per 512-row output chunk.
            accs = [ps.tile([1, FD], f32, name=f"acc{b}") for b in range(NBANKS)]
            # xv[T, p, j, r] = xT row (T*128 + p)*J + j -- partition p holds
            # J consecutive dram rows, i.e. one contiguous J*8 KiB line.
            # w_pk is packed on the host to the same (p, j) -> n mapping.
            xv = xT.rearrange("(T p j) r -> T p j r", p=P, j=J)
            NT = NBLK // J
            RH = ROWS_PER_CORE // 2
            for T in range(NT):
                # Single HWDGE ring for the whole x stream: splitting across
                # the SP+Act rings was tried and regressed 18% (packet-level
                # round-robin over the shared 16 SDMA engines + out-of-order
                # tile completion vs the PE's in-order consumption).
                xt = xp.tile([P, J, ROWS_PER_CORE], f16)
                if TAIL_SPLIT and T == NT - 1:
                    # Row-halves: banks 0..3 only need the first half, so
                    # their stop-matmuls (and copies) overlap half B's DMA.
                    nc.sync.dma_start(out=xt[:, :, :RH], in_=xv[T][:, :, :RH])
                    nc.sync.dma_start(out=xt[:, :, RH:], in_=xv[T][:, :, RH:])
                    for b in range(NBANKS):
                        for j in range(J):
                            t = T * J + j
                            nc.tensor.matmul(
                                accs[b],
                                lhsT=w_sb[:, t : t + 1],
                                rhs=xt[:, j, b * FD : (b + 1) * FD],
                                start=(t == 0),
                                stop=(t == NBLK - 1),
                            )
                        # Copy as soon as this bank's accumulation stops,
                        # alternating engines so copies pipeline 2-wide.
                        eng = nc.vector if b % 2 else nc.scalar
                        if b % 2:
                            eng.tensor_copy(
                                y_sb[:, b * FD : (b + 1) * FD], accs[b]
                            )
                        else:
                            eng.copy(
                                out=y_sb[:, b * FD : (b + 1) * FD], in_=accs[b]
                            )
                    continue
                nc.sync.dma_start(out=xt, in_=xv[T])
                for j in range(J):
                    t = T * J + j
                    for b in range(NBANKS):
                        # acc[b][0, r] += sum_p w_pk[p, t] * xt[p, j, b*FD+r]
                        nc.tensor.matmul(
                            accs[b],
                            lhsT=w_sb[:, t : t + 1],
                            rhs=xt[:, j, b * FD : (b + 1) * FD],
                            start=(t == 0),
                            stop=(t == NBLK - 1),
                        )
            if not TAIL_SPLIT:
                for b in range(NBANKS):
                    nc.scalar.copy(out=y_sb[:, b * FD : (b + 1) * FD], in_=accs[b])
            nc.sync.dma_start(out=y_out, in_=y_sb)

    nc.compile()
    _compiled_nc = nc
    return nc


def _dct_weight(dw_w):
    """w = C @ dw_w in float64, where C is the [N, K] ortho DCT-II basis."""
    n = np.arange(N, dtype=np.float64)
    k = np.arange(K, dtype=np.float64)
    C = np.cos(np.pi * (2.0 * n[:, None] + 1.0) * k[None, :] / (2.0 * N))
    C *= math.sqrt(2.0 / N)
    C[:, 0] *= 1.0 / math.sqrt(2.0)
    return (C @ np.asarray(dw_w, dtype=np.float64)).astype(np.float32)


def _erf(x):
    try:
        from scipy.special import erf

        return erf(x)
    except Exception:
        return np.vectorize(math.erf)(x).astype(x.dtype)


def _run_device(inputs, trace=False, **spmd_kwargs):
    """Run the dot-product phase on the 8 cores; return att [B, D] (pre-BN,
    pre-bias) and the BassKernelResults (for profiling from harnesses)."""
    x = np.asarray(inputs["x"])
    w = _dct_weight(inputs["dw_w"])
    w16 = (w * np.float32(W_SCALE)).astype(np.float16)
    # w_pk[p, T*J + j] = w[(T*128 + p)*J + j], matching the xv interleave.
    J = INTERLEAVE
    w_pk = np.ascontiguousarray(
        w16.reshape(NBLK // J, P, J).transpose(1, 0, 2).reshape(P, NBLK)
    )

    nc = _build()
    b_per_core = B // N_CORES
    in_maps = []
    for c in range(N_CORES):
        xs = x[c * b_per_core : (c + 1) * b_per_core].reshape(ROWS_PER_CORE, N)
        xTc = np.ascontiguousarray(xs.astype(np.float16).T)  # [N, rows]
        in_maps.append({"xT": xTc, "w_pk": w_pk})

    res = bass_utils.run_bass_kernel_spmd(
        nc, in_maps, core_ids=list(range(N_CORES)), trace=trace, **spmd_kwargs
    )
    att = np.concatenate(
        [res.results[c]["y_out"].reshape(-1) for c in range(N_CORES)]
    )
    att = (att.astype(np.float32) / np.float32(W_SCALE)).reshape(B, D)
    return att, res


def _postprocess(att, inputs):
    """Host tail on the tiny [B, D] array: +dw_b, BatchNorm (global batch
    stats, training mode), exact GELU, 1x1 conv affine, softmax over D."""
    dw_b = np.float32(np.asarray(inputs["dw_b"]).reshape(-1)[0])
    gamma = np.float32(np.asarray(inputs["gamma"]).reshape(-1)[0])
    beta = np.float32(np.asarray(inputs["beta"]).reshape(-1)[0])
    conv_w = np.float32(np.asarray(inputs["conv_w"]).reshape(-1)[0])
    conv_b = np.float32(np.asarray(inputs["conv_b"]).reshape(-1)[0])

    att = att.astype(np.float32) + dw_b
    mean = att.mean(dtype=np.float64)
    var = np.mean((att.astype(np.float64) - mean) ** 2)
    inv_std = np.float32(1.0 / math.sqrt(var + BN_EPS))
    att = (att - np.float32(mean)) * inv_std * gamma + beta
    # Exact GELU: x * 0.5 * (1 + erf(x / sqrt(2)))
    att = (att * 0.5 * (1.0 + _erf(att / np.float32(math.sqrt(2.0))))).astype(
        np.float32
    )
    att1 = att * conv_w + conv_b
    att1 = att1 - att1.max(axis=-1, keepdims=True)
    e = np.exp(att1.astype(np.float32))
    att1 = (e / e.sum(axis=-1, keepdims=True)).astype(np.float32)
    att1 = att1[:, :, None]
    return att1, (np.float32(1.0) - att1).astype(np.float32)


def kernel(**inputs):
    att, _ = _run_device(inputs)
    return _postprocess(att, inputs)
